# revision 8
# baseline (speedup 1.0000x reference)
"""ANFIS Trainium2 kernel (8 NeuronCores, Bass/Tile).

Math (reference):
  mfs[b,i,j] = exp(-(x[b,i]-centers[i,j])^2 / (2*widths[i,j]^2))   [1024,8,4]
  w[b,r]     = prod_i mfs[b,i,idx_i(r)]    r in [0, 4^8=65536), i0 slowest
  w        <- w / sum_r w
  out[b,n]   = sum_r w[b,r] * ([x[b],1] . rule_params[r,:,n])      [1024,16]

Structure: w = wA (x) wB with wA over dims 0..2 (64 vals, split 8 rA per
core) and wB over dims 3..7 (1024 vals); r = rA*1024 + rB.  Denominator
factorizes: sum_r w = prod_i (sum_j mfs[b,i,j]).

Per core:  psum[b, rA, i*16+n] = sum_rB wB[b,rB] rp[rA*1024+rB, i*16+n]
(bf16 matmuls, rB contracted on partitions, kt = 8 k-tiles), evacuated as
psum * G with G[b, rA*9+i] = wA[b,rA]/denom[b] * xb[b,i], tree-summed over
rA and strided-reduced over i.  Core partials summed on host.

v4 schedule notes:
  - wB^T via XBAR DMA transposes (dma_start_transpose) for b-tiles 1..7;
    b-tile 0 on the PE (identity matmul) to dodge the XBAR's ~2.5us
    trigger+sem latency on the pipeline head.
  - The critical-path input slab (x + membership constants) is its own
    DRAM param + SBUF tile and the FIRST sync-queue DMA: the tile dep
    tracker is conservative with strided APs, so any tile written by two
    DMAs makes early readers wait for the later DMA.
  - b-tile-0 membership chain runs in separate small tiles ahead of the
    bulk chain; one exp per chain on ACT (scale=-1 folds the sign).
  - PE warm-up dummies (zeroed tile, memset on DVE) release the HAM clock
    gate; add_dep_helper chains all DVE ops so the scheduler cannot
    starve the head of the pipeline.
  - j-scales for bt0..2 on DVE (bf16 rate), bt3..7 on ACT (1.2 GHz,
    490ns/op — fine off the critical path).  Evac xsc/tree in bf16;
    the last b-tile uses a group-local pair tree so only ~1.3us of DVE
    work remains after the final matmul.  out DMAs: bt0..6 on sync after
    the XBARs, bt7 on scalar (no queue backlog at the tail).
"""

import sys

sys.path.insert(0, "/opt/trn_rl_repo")

import numpy as np

import concourse.bacc as bacc
import concourse.tile as tile
import concourse.mybir as mybir
from concourse.ap import AP
from concourse.bass_utils import run_bass_kernel_spmd


F32 = mybir.dt.float32
BF16 = mybir.dt.bfloat16
MULT = mybir.AluOpType.mult
ADD = mybir.AluOpType.add
SUB = mybir.AluOpType.subtract
EXP = mybir.ActivationFunctionType.Exp
AXX = mybir.AxisListType.X

N_CORES = 8
B = 1024
BT = 8          # batch tiles of 128
D = 8           # input dims
DX = D + 1      # xb width (x plus ones column)
M = 4           # membership fns per dim
NO = 16         # outputs
C = DX * NO                 # 144
NRA = 64        # 4^3 (dims 0..2)
RA_LOC = NRA // N_CORES     # 8 local rA per core
NRB = 1024      # 4^5 (dims 3..7)
KT = 8          # rB partition tiles of 128
GROUPS = [(0, 3), (3, 3), (6, 2)]
SC = RA_LOC * C  # 1152
DM = D * M       # 32

N_WARM = 10             # dummy warm-up matmuls (256 cols each)

# small1 (critical): xab + centers + 1/(2w^2);  small2: wA-chain inputs
O_XAB = 0
O_CB = O_XAB + BT * DX            # 72
O_CW2N = O_CB + DM                # 104
NSM1 = O_CW2N + DM                # 136
O_XA3 = 0
O_CA3 = O_XA3 + BT * RA_LOC * 3   # 192
O_NWA2 = O_CA3 + RA_LOC * 3       # 216
NSM2 = O_NWA2 + RA_LOC * 3        # 240


def _v(t, off, dims):
    """Custom free-dim view of a [128, F] SBUF tile AP."""
    part = list(t.ap[0])
    return AP(
        tensor=t.tensor,
        offset=t.offset + off,
        ap=[part] + [[s, n] for (s, n) in dims],
    )


def build_nc():
    nc = bacc.Bacc("TRN2", target_bir_lowering=False, debug=False,
                   num_devices=N_CORES)

    small1_d = nc.declare_dram_parameter("small1", [128, NSM1], F32,
                                         isOutput=False)
    small2_d = nc.declare_dram_parameter("small2", [128, NSM2], F32,
                                         isOutput=False)
    eye_d = nc.declare_dram_parameter("eye", [128, 128], BF16, isOutput=False)
    rp_d = nc.declare_dram_parameter("rp", [128, KT * SC], BF16, isOutput=False)
    out_d = nc.declare_dram_parameter("out", [B, NO], F32, isOutput=True)

    with tile.TileContext(nc) as tc:
        with (
            tc.tile_pool(name="const", bufs=1) as cpool,
            tc.tile_pool(name="rp", bufs=1) as rppool,
            tc.tile_pool(name="wbt", bufs=1) as wbtpool,
            tc.tile_pool(name="work", bufs=2) as work,
            tc.tile_pool(name="w3s", bufs=3) as w3spool,
            tc.tile_pool(name="psD", bufs=1, space="PSUM") as psDp,
            tc.tile_pool(name="evac", bufs=3) as evpool,
            tc.tile_pool(name="ps0", bufs=2, space="PSUM") as ps0p,
            tc.tile_pool(name="ps1", bufs=2, space="PSUM") as ps1p,
            tc.tile_pool(name="ps2", bufs=2, space="PSUM") as ps2p,
        ):
            # ---- input DMAs ----
            small1 = cpool.tile([128, NSM1], F32, tag="small1")
            small2 = cpool.tile([128, NSM2], F32, tag="small2")
            eye = cpool.tile([128, 128], BF16, tag="eye")
            rp = rppool.tile([128, KT * SC], BF16, tag="rp")
            zs = cpool.tile([128, 512], BF16, tag="zs")

            nc.sync.dma_start(small1[:], small1_d[:])
            nc.sync.dma_start(eye[:], eye_d[:])
            # rp chunks: kt0/kt1 + small2 on scalar, kt2-4 sync, kt5-7 gpsimd
            nc.scalar.dma_start(rp[:, 0:SC], rp_d[:, 0:SC])
            nc.scalar.dma_start(rp[:, SC:2 * SC], rp_d[:, SC:2 * SC])
            nc.scalar.dma_start(small2[:], small2_d[:])
            for kt, eng in ((2, nc.sync), (3, nc.sync), (4, nc.sync),
                            (5, nc.gpsimd), (6, nc.gpsimd), (7, nc.gpsimd)):
                eng.dma_start(rp[:, kt * SC:(kt + 1) * SC],
                              rp_d[:, kt * SC:(kt + 1) * SC])

            xab = small1[:, O_XAB:O_XAB + BT * DX]
            cb = small1[:, O_CB:O_CB + DM]
            cw2n = small1[:, O_CW2N:O_CW2N + DM]
            xA3 = small2[:, O_XA3:O_XA3 + BT * RA_LOC * 3]
            cA3 = small2[:, O_CA3:O_CA3 + RA_LOC * 3]
            nwA2 = small2[:, O_NWA2:O_NWA2 + RA_LOC * 3]

            # ---- PE warm-up: zero tile (DVE memset, no deps) + dummies ----
            nc.vector.memset(zs[:], 0)
            psD = [psDp.tile([128, 512], F32, tag="psD0", name="psD0"),
                   psDp.tile([128, 512], F32, tag="psD1", name="psD1")]
            for i in range(N_WARM):
                nc.tensor.matmul(psD[i % 2][:, 0:256], zs[:, 0:128],
                                 zs[:, 0:256], start=True, stop=True)

            # DVE stage chain: force scheduler to respect emission order
            last_dve = [None]

            def dve(op_fn, *args, **kwargs):
                i = op_fn(*args, **kwargs)
                if last_dve[0] is not None:
                    tile.add_dep_helper(i.ins, last_dve[0].ins, sync=False,
                                        reason="dve stage order")
                last_dve[0] = i
                return i

            # bt0 membership chain in its own small tiles (clean DMA dep)
            mfs0 = cpool.tile([128, DM], F32, tag="mfs0")
            mfsR = cpool.tile([128, (BT - 1) * DM], F32, tag="mfsR")

            def mfs_chain(mfst, nbt, xoff, tg):
                dift = work.tile([128, nbt * DM], F32, tag="dif" + tg)
                d2t = work.tile([128, nbt * DM], F32, tag="d2" + tg)
                d2st = work.tile([128, nbt * DM], F32, tag="d2s" + tg)
                dve(nc.vector.tensor_tensor,
                    _v(dift[:], 0, [(DM, nbt), (M, D), (1, M)]),
                    _v(xab, xoff, [(DX, nbt), (1, D), (0, M)]),
                    _v(cb, 0, [(0, nbt), (M, D), (1, M)]),
                    op=SUB)
                dve(nc.vector.tensor_tensor,
                    d2t[:], dift[:], dift[:], op=MULT)
                dve(nc.vector.tensor_tensor,
                    _v(d2st[:], 0, [(DM, nbt), (1, DM)]),
                    _v(d2t[:], 0, [(DM, nbt), (1, DM)]),
                    _v(cw2n, 0, [(0, nbt), (1, DM)]), op=MULT)
                nc.scalar.activation(mfst[:], d2st[:], EXP, scale=-1.0)

            # mfs column views per bt (bt0 in mfs0, rest in mfsR)
            def mfs_at(bt, off):
                if bt == 0:
                    return mfs0, off
                return mfsR, (bt - 1) * DM + off

            w34 = work.tile([128, BT * 16], BF16, tag="w34")
            w56 = work.tile([128, BT * 16], BF16, tag="w56")
            w3456 = cpool.tile([128, BT * 256], BF16, tag="w3456")

            def w_chain(off, nbt):
                mfst, moff = mfs_at(off, 0)
                dve(nc.vector.tensor_tensor,
                    _v(w34[:], off * 16, [(16, nbt), (M, M), (1, M)]),
                    _v(mfst[:], moff + 3 * M, [(DM, nbt), (1, M), (0, M)]),
                    _v(mfst[:], moff + 4 * M, [(DM, nbt), (0, M), (1, M)]),
                    op=MULT)
                dve(nc.vector.tensor_tensor,
                    _v(w56[:], off * 16, [(16, nbt), (M, M), (1, M)]),
                    _v(mfst[:], moff + 5 * M, [(DM, nbt), (1, M), (0, M)]),
                    _v(mfst[:], moff + 6 * M, [(DM, nbt), (0, M), (1, M)]),
                    op=MULT)
                dve(nc.vector.tensor_tensor,
                    _v(w3456[:], off * 256, [(256, nbt), (16, 16), (1, 16)]),
                    _v(w34[:], off * 16, [(16, nbt), (1, 16), (0, 16)]),
                    _v(w56[:], off * 16, [(16, nbt), (0, 16), (1, 16)]),
                    op=MULT)

            wbt = wbtpool.tile([128, KT * B], BF16, tag="wbt")

            def jscales(bt, on_dve):
                w3sall = w3spool.tile([128, 1024], BF16, tag="w3s",
                                      name="w3sall")
                mfst, moff = mfs_at(bt, 7 * M)
                for j in range(M):
                    dst = w3sall[:, j * 256:(j + 1) * 256]
                    src = w3456[:, bt * 256:(bt + 1) * 256]
                    sc = mfst[:, moff + j: moff + j + 1]
                    if on_dve:
                        dve(nc.vector.tensor_scalar_mul, dst, src, sc)
                    else:
                        nc.scalar.mul(dst, src, sc)
                return w3sall

            # ---- S1: bt0 chain -> PE transposes ----
            mfs_chain(mfs0, 1, 0, "0")
            w_chain(0, 1)
            w3s0 = jscales(0, on_dve=True)
            psD = [psDp.tile([128, 512], F32, tag="psD0", name="psD0"),
                   psDp.tile([128, 512], F32, tag="psD1", name="psD1")]
            for j in range(M):
                for qh in range(2):
                    kt = 2 * j + qh
                    m, t = kt // 4, kt % 4
                    nc.tensor.matmul(
                        psD[m][:, t * 128:(t + 1) * 128],
                        w3s0[:, kt * 128:(kt + 1) * 128], eye[:],
                        start=True, stop=True)
            # psD evac split so main kt0 can start as early as possible
            nc.scalar.copy(_v(wbt[:], 0, [(B, 1), (1, 128)]),
                           psD[0][:, 0:128])
            nc.scalar.copy(_v(wbt[:], B, [(B, 3), (1, 128)]),
                           psD[0][:, 128:512])
            nc.scalar.copy(_v(wbt[:], 4 * B, [(B, 4), (1, 128)]),
                           psD[1][:])

            # ---- S2: bulk mfs chain (bt1..7) ----
            mfs_chain(mfsR, BT - 1, DX, "R")

            # ---- S3: bt1/bt2 w-chains + j-scales -> XBAR ----
            for bt in (1, 2):
                w_chain(bt, 1)
                w3sb = jscales(bt, on_dve=True)
                nc.sync.dma_start_transpose(
                    _v(wbt[:], bt * 128, [(B, KT), (1, 128)]), w3sb[:])

            # ---- main matmuls emitted per bt; builds interleave ----
            def main_mms(bt, ps):
                for kt in range(KT):
                    lhsT = wbt[:, kt * B + bt * 128: kt * B + (bt + 1) * 128]
                    for g, (r0, nr) in enumerate(GROUPS):
                        nc.tensor.matmul(
                            ps[g][:], lhsT,
                            _v(rp[:], (kt * RA_LOC + r0) * C,
                               [(C, nr), (1, C)]),
                            start=(kt == 0), stop=(kt == KT - 1))

            def alloc_ps():
                return [
                    ps0p.tile([128, GROUPS[0][1] * C], F32, tag="ps0", name="ps0"),
                    ps1p.tile([128, GROUPS[1][1] * C], F32, tag="ps1", name="ps1"),
                    ps2p.tile([128, GROUPS[2][1] * C], F32, tag="ps2", name="ps2")]

            ps_bt = [None] * BT
            ps_bt[0] = alloc_ps()
            main_mms(0, ps_bt[0])

            # ---- S4: bulk w-chain bt3..7 ----
            w_chain(3, BT - 3)

            # ---- S5: wA chain ----
            NA = BT * RA_LOC * 3  # 192
            dA = work.tile([128, NA], F32, tag="dA")
            dve(nc.vector.tensor_tensor,
                dA[:], xA3, _v(cA3, 0, [(0, BT), (1, RA_LOC * 3)]), op=SUB)
            d2A = work.tile([128, NA], F32, tag="d2A")
            dve(nc.vector.tensor_tensor, d2A[:], dA[:], dA[:], op=MULT)
            d2sA = work.tile([128, NA], F32, tag="d2sA")
            dve(nc.vector.tensor_tensor,
                d2sA[:], d2A[:], _v(nwA2, 0, [(0, BT), (1, RA_LOC * 3)]),
                op=MULT)
            eA = work.tile([128, BT * RA_LOC], F32, tag="eA")
            dve(nc.vector.reduce_sum,
                eA[:], _v(d2sA[:], 0, [(3, BT * RA_LOC), (1, 3)]), axis=AXX)
            wA = cpool.tile([128, BT * RA_LOC], F32, tag="wA")
            nc.scalar.activation(wA[:], eA[:], EXP, scale=-1.0)

            # ---- S6: j-scales bt3..7 on ACT + XBARs ----
            for bt in range(3, BT):
                w3sb = jscales(bt, on_dve=False)
                nc.sync.dma_start_transpose(
                    _v(wbt[:], bt * 128, [(B, KT), (1, 128)]), w3sb[:])

            # ---- S7: denominator + wAn + G ----
            s = work.tile([128, BT * D], F32, tag="s")
            dve(nc.vector.reduce_sum,
                s[:, 0:D], _v(mfs0[:], 0, [(M, D), (1, M)]), axis=AXX)
            dve(nc.vector.reduce_sum,
                s[:, D:BT * D], _v(mfsR[:], 0, [(M, (BT - 1) * D), (1, M)]),
                axis=AXX)
            p1 = work.tile([128, BT * 4], F32, tag="p1")
            dve(nc.vector.tensor_tensor,
                p1[:], _v(s[:], 0, [(D, BT), (1, 4)]),
                _v(s[:], 4, [(D, BT), (1, 4)]), op=MULT)
            p2 = work.tile([128, BT * 2], F32, tag="p2")
            dve(nc.vector.tensor_tensor,
                p2[:], _v(p1[:], 0, [(4, BT), (1, 2)]),
                _v(p1[:], 2, [(4, BT), (1, 2)]), op=MULT)
            p3 = work.tile([128, BT], F32, tag="p3")
            dve(nc.vector.tensor_tensor,
                p3[:], _v(p2[:], 0, [(2, BT)]), _v(p2[:], 1, [(2, BT)]),
                op=MULT)
            invd = cpool.tile([128, BT], F32, tag="invd")
            dve(nc.vector.reciprocal, invd[:], p3[:])
            wAn = cpool.tile([128, BT * RA_LOC], F32, tag="wAn")
            dve(nc.vector.tensor_tensor,
                wAn[:],
                _v(wA[:], 0, [(RA_LOC, BT), (1, RA_LOC)]),
                _v(invd[:], 0, [(1, BT), (0, RA_LOC)]), op=MULT)
            Gall = cpool.tile([128, BT * RA_LOC * DX], F32, tag="Gall")
            dve(nc.vector.tensor_tensor,
                Gall[:],
                _v(wAn[:], 0, [(RA_LOC, BT), (1, RA_LOC), (0, DX)]),
                _v(xab, 0, [(DX, BT), (0, RA_LOC), (1, DX)]), op=MULT)

            # ---- S8: remaining mains + evacs ----
            def evac(bt, ps, last):
                xsc = evpool.tile([128, SC], BF16, tag="xsc")
                xmul = []
                for g, (r0, nr) in enumerate(GROUPS):
                    xmul.append(dve(
                        nc.vector.tensor_tensor,
                        xsc[:, r0 * C:(r0 + nr) * C], ps[g][:],
                        _v(Gall[:], bt * RA_LOC * DX + r0 * DX,
                           [(DX, nr), (1, DX), (0, NO)]),
                        op=MULT))
                th3 = evpool.tile([128, C], BF16, tag="th3")
                ob = evpool.tile([128, NO], F32, tag="ob")
                if not last:
                    th = evpool.tile([128, 4 * C], BF16, tag="th")
                    dve(nc.vector.tensor_tensor,
                        th[:], xsc[:, 0:4 * C], xsc[:, 4 * C:8 * C], op=ADD)
                    th2 = evpool.tile([128, 2 * C], BF16, tag="th2")
                    dve(nc.vector.tensor_tensor,
                        th2[:], th[:, 0:2 * C], th[:, 2 * C:4 * C], op=ADD)
                    dve(nc.vector.tensor_tensor,
                        th3[:], th2[:, 0:C], th2[:, C:2 * C], op=ADD)
                else:
                    # group-local pair tree: tail after the g2 mult is only
                    # q3 + h1 + th3 + reduce (~1.3us)
                    q = evpool.tile([128, 4 * C], BF16, tag="th")
                    h = evpool.tile([128, 2 * C], BF16, tag="th2")
                    for k in range(3):
                        dve(nc.vector.tensor_tensor,
                            q[:, k * C:(k + 1) * C],
                            xsc[:, (2 * k) * C:(2 * k + 1) * C],
                            xsc[:, (2 * k + 1) * C:(2 * k + 2) * C], op=ADD)
                    dve(nc.vector.tensor_tensor,
                        h[:, 0:C], q[:, 0:C], q[:, C:2 * C], op=ADD)
                    dve(nc.vector.tensor_tensor,
                        q[:, 3 * C:4 * C], xsc[:, 6 * C:7 * C],
                        xsc[:, 7 * C:8 * C], op=ADD)
                    dve(nc.vector.tensor_tensor,
                        h[:, C:2 * C], q[:, 2 * C:3 * C], q[:, 3 * C:4 * C],
                        op=ADD)
                    dve(nc.vector.tensor_tensor,
                        th3[:], h[:, 0:C], h[:, C:2 * C], op=ADD)
                dve(nc.vector.reduce_sum,
                    ob[:], _v(th3[:], 0, [(1, NO), (NO, DX)]), axis=AXX)
                return ob

            obs = [None] * BT
            for bt in range(1, BT):
                ps_bt[bt] = alloc_ps()
                main_mms(bt, ps_bt[bt])
                obs[bt - 1] = evac(bt - 1, ps_bt[bt - 1], last=False)
            obs[BT - 1] = evac(BT - 1, ps_bt[BT - 1], last=True)

            for bt in range(BT - 1):
                nc.sync.dma_start(out_d[bt * 128:(bt + 1) * 128, :],
                                  obs[bt][:])
            nc.scalar.dma_start(out_d[(BT - 1) * 128:BT * 128, :],
                                obs[BT - 1][:])

    nc.compile()
    return nc


_NC_CACHE = None


def _get_nc():
    global _NC_CACHE
    if _NC_CACHE is None:
        _NC_CACHE = build_nc()
    return _NC_CACHE


def _prep_in_maps(x, centers, widths, rule_params):
    import ml_dtypes

    x = np.asarray(x, np.float32)
    centers = np.asarray(centers, np.float32)
    widths = np.asarray(widths, np.float32)
    rule_params = np.asarray(rule_params, np.float32)

    # xab[p, bt*9+i] = x[bt*128+p, i] for i<8; 1.0 at i=8
    xab = np.ones((128, BT, DX), np.float32)
    xab[:, :, :D] = x.reshape(BT, 128, D).transpose(1, 0, 2)
    xab = xab.reshape(128, BT * DX)
    cb = np.broadcast_to(centers.reshape(1, DM), (128, DM))
    cw2n = np.broadcast_to((1.0 / (2.0 * widths * widths)).reshape(1, DM),
                           (128, DM))
    eye = np.eye(128, dtype=ml_dtypes.bfloat16)

    # xA3[p, bt*24 + r*3 + k] = x[bt*128+p, k]
    xA3 = np.broadcast_to(
        x.reshape(BT, 128, D).transpose(1, 0, 2)[:, :, None, 0:3],
        (128, BT, RA_LOC, 3)).reshape(128, BT * RA_LOC * 3)

    # rule_params rows r = rA*1024 + q*4 + j -> per core [p, kt, rA, c]
    # with row order rB' = j*256 + q, kt = rB' tile of 128.
    rp4 = rule_params.reshape(NRA, 256, M, C).transpose(0, 2, 1, 3)
    rp4 = rp4.reshape(NRA, NRB, C)

    in_maps = []
    for c in range(N_CORES):
        ra0 = c * RA_LOC
        idx = np.empty((RA_LOC, 3), np.int64)
        for r in range(RA_LOC):
            ra = ra0 + r
            idx[r] = [(ra >> 4) & 3, (ra >> 2) & 3, ra & 3]
        k = np.arange(3)
        cA = centers[k[None, :], idx]
        wtA = widths[k[None, :], idx]
        cA3 = np.broadcast_to(cA.reshape(1, RA_LOC * 3), (128, RA_LOC * 3))
        nwA2 = np.broadcast_to(
            (1.0 / (2.0 * wtA * wtA)).reshape(1, RA_LOC * 3),
            (128, RA_LOC * 3))
        small1 = np.ascontiguousarray(
            np.concatenate([xab, cb, cw2n], axis=1, dtype=np.float32))
        small2 = np.ascontiguousarray(
            np.concatenate([xA3, cA3, nwA2], axis=1, dtype=np.float32))

        rp_c = rp4[ra0:ra0 + RA_LOC]                     # [8, 1024, 144]
        rp_c = rp_c.reshape(RA_LOC, KT, 128, C).transpose(2, 1, 0, 3)
        rp_c = np.ascontiguousarray(
            rp_c.reshape(128, KT * SC)).astype(ml_dtypes.bfloat16)

        in_maps.append({"small1": small1, "small2": small2, "eye": eye,
                        "rp": rp_c})
    return in_maps


def kernel(x, centers, widths, rule_params, _trace=False):
    nc = _get_nc()
    in_maps = _prep_in_maps(x, centers, widths, rule_params)
    res = run_bass_kernel_spmd(nc, in_maps, core_ids=list(range(N_CORES)),
                               trace=_trace)
    out = np.sum([np.asarray(res.results[c]["out"], np.float32)
                  for c in range(N_CORES)], axis=0)
    if _trace:
        kernel._last_exec_time_ns = res.exec_time_ns
        kernel._last_results = res
    return out
